# revision 1
# baseline (speedup 1.0000x reference)
"""Trainium2 Bass kernel for ClipPairWiseLossAll.

loss = sum_{i<j} || relu(r_i - r_j) ||_2   with r = repr[GT], M=512, N=768.

Strategy (8 NeuronCores, SPMD, one shared NEFF):
  * Host: gather r = repr[GT], transpose -> rT [N=768, M=512], cast bf16.
  * Pair space decomposed by DIAGONALS: diagonal o covers pairs (t, t+o),
    t in [0, 512-o). Core c owns o in {16k + (c+1), 16k + (16-c)}, k<32 —
    511 real diagonals + 1 masked dummy, ~16.4k pairs per core.
  * The per-core shift lives in the DATA, not the program: core c receives
    rtab = [rT shifted left by c+1, rT shifted left by 16-c] so the device
    always slices at offset 16k (uniform across cores -> single NEFF).
  * Per k (two diagonals of rounded length L = 512-16k, all 6 n-chunks and
    both slots in single instructions):
      d  = rt2[., t] - rtab[., 16k+t]   one tensor_tensor sub (bf16 2x)
      E  = relu(d)                      one tensor_scalar max-imm (bf16 4x)
      E2 = E^2 -> fp8                   one ACT Square
      psum[row m] += sum_n E2           fp8 DoubleRow one-hot matmuls
  * A per-core mask kills rounded-up columns, ACT computes sqrt with a
    fused row-sum, host adds the 8x64 partials.
"""

import numpy as np

M = 512
N = 768
P = 128
NCH = N // P  # 6
NCORES = 8
NS = 64  # diagonals per core (2 per k)


def _o_list(c):
    out = []
    for k in range(32):
        out.append(16 * k + c + 1)
        out.append(16 * k + 16 - c)
    return out


_PROG = {}

# square-pass engine per k: "act" or "dve" (dve -> bf16 e2, bf16 matmuls)
SQ_DVE_KS = (0,)


def _build_program():
    if "nc" in _PROG:
        return _PROG["nc"]

    from contextlib import ExitStack

    import concourse.bass as bass
    import concourse.bacc as bacc
    import concourse.tile as tile
    from concourse import mybir

    AOT = mybir.AluOpType
    AFT = mybir.ActivationFunctionType
    bf16 = mybir.dt.bfloat16
    fp8 = mybir.dt.float8e4
    f32 = mybir.dt.float32

    nc = bacc.Bacc(
        "TRN2",
        target_bir_lowering=False,
        debug=False,
        enable_asserts=False,
        num_devices=NCORES,
    )

    rt_d = nc.dram_tensor("rt", [P, NCH * M], bf16, kind="ExternalInput")
    rtab_d = nc.dram_tensor("rtab", [P, 2 * NCH * M], bf16, kind="ExternalInput")
    oh_d = nc.dram_tensor("oh", [P, NS * 2 * NS], fp8, kind="ExternalInput")
    out_d = nc.dram_tensor("out", [NS, 1], f32, kind="ExternalOutput")

    with ExitStack() as ctx:
        tc = ctx.enter_context(tile.TileContext(nc))
        singles = ctx.enter_context(tc.tile_pool(name="singles", bufs=1))
        dpool = ctx.enter_context(tc.tile_pool(name="d", bufs=4))
        epool = ctx.enter_context(tc.tile_pool(name="e", bufs=4))
        e2pool = ctx.enter_context(tc.tile_pool(name="e2", bufs=4))
        pspool = ctx.enter_context(tc.tile_pool(name="ps", bufs=1, space="PSUM"))

        # one-hot lhsT stack first (PE needs it for the very first matmul),
        # on the GPSIMD SWDGE queue so it runs parallel to the sync-queue DMAs
        oh = singles.tile([P, NS, 2, NS], fp8)
        nc.gpsimd.dma_start(out=oh, in_=oh_d.ap())
        # piecewise rt/rtab DMAs, ordered so the first (smallest-L) compute
        # iterations can start as soon as their slices arrive
        NPC = 4
        PCM = M // NPC
        rt_sb = singles.tile([P, NCH, M], bf16)
        rt_view = rt_d.ap().rearrange("p (c t) -> p c t", c=NCH)
        rtab_sb = singles.tile([P, 2, NCH, M], bf16)
        rtab_view = rtab_d.ap().rearrange("p (s c t) -> p s c t", s=2, c=NCH)
        for pc in range(NPC):
            lo, hi = pc * PCM, (pc + 1) * PCM
            nc.sync.dma_start(out=rt_sb[:, :, lo:hi], in_=rt_view[:, :, lo:hi])
            lo2, hi2 = M - hi, M - lo
            nc.sync.dma_start(
                out=rtab_sb[:, :, :, lo2:hi2], in_=rtab_view[:, :, :, lo2:hi2]
            )

        ps = pspool.tile([NS, M], f32)
        nc.vector.memset(ps, 0.0)

        # bf16 one-hot lhsT rows for the DVE-squared k's
        ohb = singles.tile([P, 2 * len(SQ_DVE_KS), NS], bf16)
        nc.vector.memset(ohb, 0.0)
        _ohb_col = {}
        for j, kq in enumerate(SQ_DVE_KS):
            for slot in range(2):
                m = 2 * kq + slot
                jj = 2 * j + slot
                _ohb_col[m] = jj
                nc.vector.memset(ohb[:, jj, m : m + 1], 1.0)

        for k in range(31, -1, -1):
            L = M - 16 * k
            d_t = dpool.tile([P, 2, NCH, M], bf16, tag="d")
            in0s = rt_sb[:, :, 0:L]
            in0 = bass.AP(
                tensor=in0s.tensor,
                offset=in0s.offset,
                ap=[in0s.ap[0], [0, 2], in0s.ap[1], in0s.ap[2]],
            )
            nc.vector.tensor_sub(
                d_t[:, :, :, 0:L],
                in0,
                rtab_sb[:, :, :, 16 * k : 16 * k + L],
            )
            e_t = epool.tile([P, 2, NCH, M], bf16, tag="e")
            nc.vector.tensor_scalar(
                out=e_t[:, :, :, 0:L],
                in0=d_t[:, :, :, 0:L],
                scalar1=0.0,
                scalar2=None,
                op0=AOT.max,
            )
            if k in SQ_DVE_KS:
                e2b_t = e2pool.tile([P, 2, NCH, M], bf16, tag="e2b")
                nc.vector.tensor_mul(
                    e2b_t[:, :, :, 0:L], e_t[:, :, :, 0:L], e_t[:, :, :, 0:L]
                )
                for slot in range(2):
                    m = 2 * k + slot
                    for c in range(NCH):
                        nc.tensor.matmul(
                            ps[:, 0:L],
                            ohb[:, _ohb_col[m], :],
                            e2b_t[:, slot, c, 0:L],
                            start=False,
                            stop=False,
                            skip_group_check=True,
                        )
            else:
                e2_t = e2pool.tile([P, 2, NCH, M], fp8, tag="e2")
                nc.scalar.activation(
                    out=e2_t[:, :, :, 0:L],
                    in_=e_t[:, :, :, 0:L],
                    func=AFT.Square,
                )
                for slot in range(2):
                    m = 2 * k + slot
                    for c2 in range(NCH // 2):
                        nc.tensor.matmul(
                            ps[:, 0:L],
                            oh[:, m, :, :],
                            e2_t[:, slot, 2 * c2 : 2 * c2 + 2, 0:L],
                            start=False,
                            stop=False,
                            skip_group_check=True,
                            perf_mode=mybir.MatmulPerfMode.DoubleRow,
                        )

        sqrt_t = singles.tile([NS, M], bf16)
        res = singles.tile([NS, 1], f32)
        nc.scalar.activation(out=sqrt_t, in_=ps[:, :], func=AFT.Sqrt, accum_out=res)
        nc.sync.dma_start(out=out_d.ap(), in_=res)

    nc.compile()
    _PROG["nc"] = nc
    return nc


def _shift_pc(rT_bf, h):
    """rT shifted left by h columns, HUGE-padded, in [p, chunk, t] layout.

    The pad makes relu(r_t - pad) exactly 0, so rounded-up columns
    contribute nothing and no mask pass is needed."""
    N_, M_ = rT_bf.shape
    sh = np.full_like(rT_bf, 3.0e38)
    if h < M_:
        sh[:, : M_ - h] = rT_bf[:, h:]
    return np.transpose(sh.reshape(NCH, P, M_), (1, 0, 2))  # [P, NCH, M]


def _in_maps(repr_np, GT_np):
    import ml_dtypes

    r = np.asarray(repr_np, dtype=np.float32)[np.asarray(GT_np).astype(np.int64)]
    rT = np.ascontiguousarray(r.T)  # [N, M] f32
    rT_bf = rT.astype(ml_dtypes.bfloat16)

    base = np.transpose(rT_bf.reshape(NCH, P, M), (1, 0, 2))  # [P, NCH, M]
    rt = np.ascontiguousarray(base).reshape(P, -1)

    ohs = np.zeros((P, NS, 2, NS), dtype=ml_dtypes.float8_e4m3)
    for m in range(NS):
        ohs[:, m, :, m] = 1.0
    ohs = ohs.reshape(P, NS * 2 * NS)

    maps = []
    for c in range(NCORES):
        rtab = np.stack(
            [_shift_pc(rT_bf, c + 1), _shift_pc(rT_bf, 16 - c)], axis=1
        ).reshape(P, -1)
        maps.append({"rt": rt, "rtab": np.ascontiguousarray(rtab), "oh": ohs})
    return maps


def run_device(repr_np, GT_np, trace=False, trace_cores=None):
    """Run the bass kernel on 8 cores; returns (total, BassKernelResults)."""
    from concourse.bass_utils import run_bass_kernel_spmd

    nc = _build_program()
    maps = _in_maps(repr_np, GT_np)
    res = run_bass_kernel_spmd(
        nc,
        maps,
        core_ids=list(range(NCORES)),
        trace=trace,
        trace_cores=trace_cores,
    )
    total = 0.0
    for core_out in res.results:
        total += float(core_out["out"].astype(np.float64).sum())
    return np.float32(total), res


def kernel(repr, GT):
    total, _ = run_device(repr, GT, trace=False)
    return total



# revision 2
# speedup vs baseline: 1.0090x; 1.0090x over previous
"""Trainium2 Bass kernel for ClipPairWiseLossAll (v4).

loss = sum_{i<j} || relu(r_i - r_j) ||_2   with r = repr[GT], M=512, N=768.

Strategy (8 NeuronCores, SPMD, one shared NEFF):
  * Pair space split by diagonal offset o = j - i:
      - o in [1, 288] (k = o//16 < KA=18): LAYOUT A - diagonals, rT layout
        [n-chunk partitions, t free].  DVE tensor_sub (bf16 2x) + DVE relu
        (tensor_scalar 4x); square on DVE (TT mult, k in SQ_DVE_KS, bf16
        one-hot PE reduce) or on ACT (fp8 out, DoubleRow fp8 one-hot PE
        reduce).  Per-core shift carried in the DATA (rtab = shifted rT,
        HUGE-padded) -> single NEFF.
      - o in [289, 511]: TAIL - pairs bucketed by 128-row block-pair
        (bi,bj), units of 128 pairs, 2 units per group; schedule
        {(0,3)x16, (0,2)x6, (1,3)x6} is core-independent (the +c/-c
        offsets cancel).  PE fp8 DoubleRow matmuls with STATIONARY
        two-block r8 slices and STREAMED +/-1 selector columns (both
        group units fused -> FD=256) write d in layout-A shape
        [n-in-chunk partitions, (c, unit, pair) free] to PSUM.  ACT relu
        (PSUM->SBUF bf16), DVE square (TT mult 2x), then per unit two
        FD=384 one-hot matmuls accumulate chunk-pair partials into spare
        rows 36..63 of the shared 1-bank psum (chunks c and c+3 share
        columns).  A single small DVE tensor_reduce folds the 3 chunk
        columns before the tail sqrt.
  * ACT sqrt with fused row-sum (layout-A rows, then tail rows);
    host adds the per-core partials.
"""

import numpy as np

M = 512
N = 768
P = 128
NCH = N // P  # 6
NCORES = 8

KA = 18  # k < KA -> layout A; rest -> tail units
NSA = 2 * KA  # layout-A psum rows (slot-major: m = 2k+slot)
NSP = 64  # psum rows / one-hot columns

# tail unit schedule: (bi, bj) per unit; groups of 2 units share (bi,bj)
SCHED = [(0, 3)] * 16 + [(0, 2)] * 6 + [(1, 3)] * 6
NT = len(SCHED)  # 28
NG = NT // 2  # 14 groups

# layout-A squares computed on DVE (bf16) instead of ACT (fp8)
SQ_DVE_KS = (0, 1)
TOH = 2 * len(SQ_DVE_KS)

_PROG = {}


def _build_program():
    if "nc" in _PROG:
        return _PROG["nc"]

    from contextlib import ExitStack

    import concourse.bass as bass
    import concourse.bacc as bacc
    import concourse.tile as tile
    from concourse import mybir

    AOT = mybir.AluOpType
    AFT = mybir.ActivationFunctionType
    bf16 = mybir.dt.bfloat16
    fp8 = mybir.dt.float8e4
    f32 = mybir.dt.float32

    nc = bacc.Bacc(
        "TRN2",
        target_bir_lowering=False,
        debug=False,
        enable_asserts=False,
        num_devices=NCORES,
    )

    rt_d = nc.dram_tensor("rt", [P, NCH * M], bf16, kind="ExternalInput")
    rtab_d = nc.dram_tensor("rtab", [P, 2 * NCH * M], bf16, kind="ExternalInput")
    oh_d = nc.dram_tensor("oh", [P, NSA * 2 * NSP], fp8, kind="ExternalInput")
    ohb_d = nc.dram_tensor("ohb", [P, TOH * NSP], bf16, kind="ExternalInput")
    ohbt_d = nc.dram_tensor("ohbt", [P, NT * 32], bf16, kind="ExternalInput")
    r8_d = nc.dram_tensor("r8", [P, 4 * N], fp8, kind="ExternalInput")
    sel_d = nc.dram_tensor("sel", [P, 2 * NT * P], fp8, kind="ExternalInput")
    outa_d = nc.dram_tensor("outa", [NSA, 1], f32, kind="ExternalOutput")
    outb_d = nc.dram_tensor("outb", [NT, 1], f32, kind="ExternalOutput")

    with ExitStack() as ctx:
        tc = ctx.enter_context(tile.TileContext(nc))
        singles = ctx.enter_context(tc.tile_pool(name="singles", bufs=1))
        dpool = ctx.enter_context(tc.tile_pool(name="d", bufs=3))
        epool = ctx.enter_context(tc.tile_pool(name="e", bufs=3))
        e2pool = ctx.enter_context(tc.tile_pool(name="e2", bufs=3))
        espool = ctx.enter_context(tc.tile_pool(name="es", bufs=2))
        et2pool = ctx.enter_context(tc.tile_pool(name="et2", bufs=2))
        pspool = ctx.enter_context(tc.tile_pool(name="ps", bufs=1, space="PSUM"))
        dpspool = ctx.enter_context(tc.tile_pool(name="dps", bufs=2, space="PSUM"))

        # tail + one-hot inputs on the gpsimd/scalar DMA queues so the
        # tail can start while the sync queue still streams rt/rtab
        r8 = singles.tile([P, 4, N], fp8)
        nc.gpsimd.dma_start(out=r8, in_=r8_d.ap())
        sel = singles.tile([P, 2, NT * P], fp8)
        sel_view = sel_d.ap().rearrange("p (s q) -> p s q", s=2)
        nc.gpsimd.dma_start(out=sel[:, :, : 8 * P], in_=sel_view[:, :, : 8 * P])
        ohbt = singles.tile([P, NT, 32], bf16)
        nc.gpsimd.dma_start(out=ohbt, in_=ohbt_d.ap())
        ohb = singles.tile([P, TOH, NSP], bf16)
        nc.gpsimd.dma_start(out=ohb, in_=ohb_d.ap())
        nc.gpsimd.dma_start(out=sel[:, :, 8 * P :], in_=sel_view[:, :, 8 * P :])
        oh = singles.tile([P, NSA, 2, NSP], fp8)
        nc.gpsimd.dma_start(out=oh, in_=oh_d.ap())

        # piecewise rt/rtab DMAs on the sync queue, ordered to unblock the
        # first worklist items (k=17/16 halves) as early as possible
        rt_sb = singles.tile([P, NCH, M], bf16)
        rt_view = rt_d.ap().rearrange("p (c t) -> p c t", c=NCH)
        rtab_sb = singles.tile([P, 2, NCH, M], bf16)
        rtab_view = rtab_d.ap().rearrange("p (s c t) -> p s c t", s=2, c=NCH)

        def _rt(lo, hi):
            nc.sync.dma_start(out=rt_sb[:, :, lo:hi], in_=rt_view[:, :, lo:hi])

        def _rtab(lo, hi):
            nc.sync.dma_start(
                out=rtab_sb[:, :, :, lo:hi], in_=rtab_view[:, :, :, lo:hi]
            )

        _rt(0, 120)
        _rtab(272, 392)  # k17 h1
        _rtab(392, 512)  # k17 h2 (rt 120:240 next)
        _rt(120, 256)
        _rtab(256, 272)  # k16 both halves now have rtab [256:512]
        _rtab(176, 256)  # k15..k11 slices
        _rt(256, 384)
        _rtab(80, 176)
        _rt(384, 512)
        _rtab(0, 80)

        ps = pspool.tile([NSP, M], f32)
        nc.vector.memset(ps, 0.0)
        ps2 = pspool.tile([32, 384], f32)
        nc.vector.memset(ps2, 0.0)

        def emit_tail_group(g):
            """2 tail units (shared bi,bj): d -> psum [n-in-chunk part,
            (c, unit, pair) free], relu+square; returns a closure that
            emits the one-hot reduce MMs (deferred to destagger the PE
            queue from the ACT/DVE pipeline)."""
            bi, bj = SCHED[2 * g]
            st = bj - bi
            dps = dpspool.tile([P, NCH * 256], f32, tag="dps")
            for c in range(NCH):
                nc.tensor.matmul(
                    dps[:, c * 256 : (c + 1) * 256],
                    r8[:, bi : bj + 1 : st, c * P : (c + 1) * P],
                    sel[:, :, 2 * g * P : (2 * g + 2) * P],
                    start=True,
                    stop=True,
                    skip_group_check=True,
                    perf_mode=mybir.MatmulPerfMode.DoubleRow,
                )
            e_sb = espool.tile([P, NCH * 256], bf16, tag="es")
            nc.scalar.activation(out=e_sb, in_=dps, func=AFT.Relu)
            e2_sb = et2pool.tile([P, NCH, 2, P], bf16, tag="e2t")
            nc.vector.tensor_mul(
                e2_sb,
                e_sb.rearrange("p (c v q) -> p c v q", c=NCH, v=2),
                e_sb.rearrange("p (c v q) -> p c v q", c=NCH, v=2),
            )

            def onehots():
                for v in range(2):
                    u = 2 * g + v
                    for ch in range(2):
                        nc.tensor.matmul(
                            ps2[:, :],
                            ohbt[:, u, :],
                            e2_sb[:, 3 * ch : 3 * ch + 3, v, :],
                            start=False,
                            stop=False,
                            skip_group_check=True,
                        )

            return onehots

        def emit_layout_a(k, t0, t1, mms=True):
            """One column-slice [t0, t1) of diagonal band k (t1 <= L).
            mms=False skips the one-hot matmuls (caller emits them for
            the full width later)."""
            d_t = dpool.tile([P, 2, NCH, M], bf16, tag="d")
            in0s = rt_sb[:, :, t0:t1]
            in0 = bass.AP(
                tensor=in0s.tensor,
                offset=in0s.offset,
                ap=[in0s.ap[0], [0, 2], in0s.ap[1], in0s.ap[2]],
            )
            nc.vector.tensor_sub(
                d_t[:, :, :, t0:t1],
                in0,
                rtab_sb[:, :, :, 16 * k + t0 : 16 * k + t1],
            )
            e_t = epool.tile([P, 2, NCH, M], bf16, tag="e")
            nc.vector.tensor_scalar(
                out=e_t[:, :, :, t0:t1],
                in0=d_t[:, :, :, t0:t1],
                scalar1=0.0,
                scalar2=None,
                op0=AOT.max,
            )
            if k in SQ_DVE_KS:
                e2b_t = e2pool.tile([P, 2, NCH, M], bf16, tag="e2b")
                nc.vector.tensor_mul(
                    e2b_t[:, :, :, t0:t1], e_t[:, :, :, t0:t1], e_t[:, :, :, t0:t1]
                )
                if not mms:
                    return e2b_t
                for slot in range(2):
                    m = 2 * k + slot
                    for c in range(NCH):
                        nc.tensor.matmul(
                            ps[:, t0:t1],
                            ohb[:, _ohb_col[m], :],
                            e2b_t[:, slot, c, t0:t1],
                            start=False,
                            stop=False,
                            skip_group_check=True,
                        )
            else:
                e2_t = e2pool.tile([P, 2, NCH, M], fp8, tag="e2")
                nc.scalar.activation(
                    out=e2_t[:, :, :, t0:t1],
                    in_=e_t[:, :, :, t0:t1],
                    func=AFT.Square,
                )
                for slot in range(2):
                    m = 2 * k + slot
                    for c2 in range(NCH // 2):
                        nc.tensor.matmul(
                            ps[:, t0:t1],
                            oh[:, m, :, :],
                            e2_t[:, slot, 2 * c2 : 2 * c2 + 2, t0:t1],
                            start=False,
                            stop=False,
                            skip_group_check=True,
                            perf_mode=mybir.MatmulPerfMode.DoubleRow,
                        )

        _ohb_col = {}
        for j, kq in enumerate(SQ_DVE_KS):
            for slot in range(2):
                _ohb_col[2 * kq + slot] = 2 * j + slot

        def tail_finalize():
            # fold the 3 chunk-pair columns, then sqrt + fused row sum
            tin = ps2[0:NT, :]
            tin3 = bass.AP(
                tensor=tin.tensor,
                offset=tin.offset,
                ap=[tin.ap[0], [1, P], [P, 3]],
            )
            tsum = singles.tile([NT, P], f32)
            nc.vector.tensor_reduce(
                out=tsum, in_=tin3, op=AOT.add, axis=mybir.AxisListType.X
            )
            sqrt_b = singles.tile([NT, P], bf16)
            res_b = singles.tile([NT, 1], f32)
            nc.scalar.activation(
                out=sqrt_b, in_=tsum, func=AFT.Sqrt, accum_out=res_b
            )
            nc.sync.dma_start(out=outb_d.ap(), in_=res_b)

        # worklist: layout-A items (k=17,16 halved for early start; k=1,0
        # split for a short final ladder) interleaved with tail groups.
        # Tail one-hot MMs are deferred one step to destagger the PE FIFO.
        a_items = [
            (17, 0, 120),
            (17, 120, 240),
            (16, 0, 128),
            (16, 128, 256),
        ] + [(k, 0, M - 16 * k) for k in range(KA - 3, 1, -1)]
        gi = ki = 0
        pending = []
        NA = len(a_items)
        while ki < NA or gi < NG:
            emit_g = gi < NG and (ki >= NA or gi * NA <= ki * NG)
            if emit_g:
                pending.append(emit_tail_group(gi))
                gi += 1
            else:
                emit_layout_a(*a_items[ki])
                ki += 1
            if len(pending) > 1:
                pending.pop(0)()
        for fn in pending:
            fn()
        tail_finalize()
        emit_layout_a(1, 0, 256)
        emit_layout_a(1, 256, 496)
        for q in range(4):
            emit_layout_a(0, 128 * q, 128 * (q + 1))

        # layout-A sqrt + fused row sum
        sqrt_a = singles.tile([NSA, M], bf16)
        res_a = singles.tile([NSA, 1], f32)
        nc.scalar.activation(
            out=sqrt_a, in_=ps[0:NSA, :], func=AFT.Sqrt, accum_out=res_a
        )
        nc.sync.dma_start(out=outa_d.ap(), in_=res_a)

    nc.compile()
    _PROG["nc"] = nc
    return nc


def _shift_pc(rT_bf, h):
    """rT shifted left by h columns, HUGE-padded, in [p, chunk, t] layout."""
    N_, M_ = rT_bf.shape
    sh = np.full_like(rT_bf, 3.0e38)
    if h < M_:
        sh[:, : M_ - h] = rT_bf[:, h:]
    return np.transpose(sh.reshape(NCH, P, M_), (1, 0, 2))  # [P, NCH, M]


def _tail_pairs(c):
    """Core c's tail pairs bucketed by (bi, bj)."""
    buckets = {(0, 3): [], (0, 2): [], (1, 3): []}
    for k in range(KA, 32):
        for o in (16 * k + c + 1, 16 * k + 16 - c):
            if o >= M:
                continue
            for t in range(M - o):
                buckets[(t // P, (t + o) // P)].append((t, t + o))
    return buckets


def _in_maps(repr_np, GT_np):
    import ml_dtypes

    r = np.asarray(repr_np, dtype=np.float32)[np.asarray(GT_np).astype(np.int64)]
    rT = np.ascontiguousarray(r.T)  # [N, M] f32
    rT_bf = rT.astype(ml_dtypes.bfloat16)

    base = np.transpose(rT_bf.reshape(NCH, P, M), (1, 0, 2))  # [P, NCH, M]
    rt = np.ascontiguousarray(base).reshape(P, -1)

    ohs = np.zeros((P, NSA, 2, NSP), dtype=ml_dtypes.float8_e4m3)
    for m in range(NSA):
        ohs[:, m, :, m] = 1.0
    ohs = ohs.reshape(P, NSA * 2 * NSP)

    ohbs = np.zeros((P, TOH, NSP), dtype=ml_dtypes.bfloat16)
    for j, kq in enumerate(SQ_DVE_KS):
        for slot in range(2):
            ohbs[:, 2 * j + slot, 2 * kq + slot] = 1.0
    ohbs = ohbs.reshape(P, TOH * NSP)

    ohbts = np.zeros((P, NT, 32), dtype=ml_dtypes.bfloat16)
    for u in range(NT):
        ohbts[:, u, u] = 1.0
    ohbts = ohbts.reshape(P, NT * 32)

    # row-major fp8 blocks: r8[p, b, n] = r[b*128 + p, n]
    r8 = (
        r.reshape(4, P, N).transpose(1, 0, 2).astype(ml_dtypes.float8_e4m3)
    ).reshape(P, 4 * N)
    r8 = np.ascontiguousarray(r8)

    maps = []
    for c in range(NCORES):
        rtab = np.stack(
            [_shift_pc(rT_bf, c + 1), _shift_pc(rT_bf, 16 - c)], axis=1
        ).reshape(P, -1)

        selm = np.zeros((P, 2, NT * P), dtype=ml_dtypes.float8_e4m3)
        buckets = _tail_pairs(c)
        u = 0
        for bb, cnt in (((0, 3), 16), ((0, 2), 6), ((1, 3), 6)):
            prs = buckets[bb]
            bi, bj = bb
            assert len(prs) <= cnt * P, (c, bb, len(prs))
            for uu in range(cnt):
                for m, (i, j) in enumerate(prs[uu * P : (uu + 1) * P]):
                    selm[i - P * bi, 0, (u + uu) * P + m] = 1.0
                    selm[j - P * bj, 1, (u + uu) * P + m] = -1.0
            u += cnt
        maps.append(
            {
                "rt": rt,
                "rtab": np.ascontiguousarray(rtab),
                "oh": ohs,
                "ohb": ohbs,
                "ohbt": ohbts,
                "r8": r8,
                "sel": np.ascontiguousarray(selm.reshape(P, -1)),
            }
        )
    return maps


def run_device(repr_np, GT_np, trace=False, trace_cores=None):
    """Run the bass kernel on 8 cores; returns (total, BassKernelResults)."""
    from concourse.bass_utils import run_bass_kernel_spmd

    nc = _build_program()
    maps = _in_maps(repr_np, GT_np)
    res = run_bass_kernel_spmd(
        nc,
        maps,
        core_ids=list(range(NCORES)),
        trace=trace,
        trace_cores=trace_cores,
    )
    total = 0.0
    for core_out in res.results:
        total += float(core_out["outa"].astype(np.float64).sum())
        total += float(core_out["outb"].astype(np.float64).sum())
    return np.float32(total), res


def kernel(repr, GT):
    total, _ = run_device(repr, GT, trace=False)
    return total
